# revision 43
# baseline (speedup 1.0000x reference)
"""Multi-head attention (B=16, S=512, H=768, NH=12) on 8 Trainium2 NeuronCores.

Strategy: data-parallel over batch - 2 batches per core, no collectives.

v2 dataflow (all matmul inputs bf16, fp32 PSUM accumulation). The kernel is
PE-bound (~95us of matmul at 2.4GHz per core), so the structure keeps the PE
issue queue dense from the first microsecond after the runtime preamble:

  - wqkv^T for q,k is pre-blocked host-side into 12 column blocks
    [128, 6*128] (one per transposed output block) so each block is a single
    contiguous 0.2MB DMA; blocks stream round-robin across the 3 DGE queues
    (sync/scalar/gpsimd) interleaved with the x chunks, and the first
    projection matmul issues ~1us after the DMA engines come up.
  - QKV projection for q,k computed transposed (qkv^T[o, s]) so per-head
    q^T/k^T land with the head dim on partitions; v in natural [s, o]
    orientation into per-head slots of width 128 whose upper 64 columns are
    ones (tile is memset to 1.0, then the v columns are overwritten) so the
    attention-value matmul also emits the softmax denominator.
  - scores^T = k^T.T @ q^T per head pair into one 2-bank PSUM tile; heads of
    a pair occupy PE row-groups 0-63/64-127 so their matmuls run
    concurrently; one wide exp per (pair, kb) on ScalarE with the 1/sqrt(dk)
    scale fused (no max-subtraction: |scores| < ~10 for these activations).
  - AV accumulates over the 4 sk blocks; PSUM rows 64..127 hold the
    denominator broadcast over 64 partitions; per-head reciprocal reads the
    denominator rows straight out of PSUM, then one multiply per head writes
    the normalized y^T block.
  - attention is ACT(exp)-bound, so the PE idle inside attention(b0) is
    filled with the whole QKV projection of batch 1, and attention(b1) is
    filled with batch 0's output projection; the q/k blocks of batch 1's
    last head pair are deferred into attention(b1) to balance it. The final
    output projection drains in two passes (heads 0-4 first, then head 5 +
    bias + store) so it overlaps the last pair's normalize chain.

attn_mask from the reference setup is all-ones; a non-trivial mask falls
back to a numpy implementation.
"""

import sys

sys.path.insert(0, "/opt/trn_rl_repo")

import numpy as np

B, S, H, NH = 16, 512, 768, 12
DK = H // NH  # 64
N_CORES = 8
NB = B // N_CORES  # batches per core = 2
KC = H // 128  # 6 contraction chunks
SBLK = S // 128  # 4 s-blocks of 128
VW = 2 * DK  # 128: per-head v slot width (64 v cols + 64 ones cols)
NORM_SAFE = True  # True: baseline den-gather normalize (more DVE time)
KORD = [0, 2, 5, 1, 3, 4]  # contraction order matched to x-chunk DMA arrival

_PROG_CACHE = {}


def _build_program():
    import concourse.tile as tile
    from concourse import bacc, mybir

    f32 = mybir.dt.float32
    cdt = mybir.dt.bfloat16
    EXP = mybir.ActivationFunctionType.Exp

    nc = bacc.Bacc("TRN2", target_bir_lowering=False, debug=False,
                   num_devices=N_CORES)

    xt_d = nc.declare_dram_parameter("xt", [NB, H, S], cdt, isOutput=False)
    wqk_d = nc.declare_dram_parameter("wqkb", [2 * KC, 128, KC * 128], cdt, isOutput=False)
    wv_d = nc.declare_dram_parameter("wvb", [128, KC * H], cdt, isOutput=False)
    wo_d = nc.declare_dram_parameter("wot", [H, H], cdt, isOutput=False)
    bqk_d = nc.declare_dram_parameter("bqk", [128, 2 * KC], f32, isOutput=False)
    bo2_d = nc.declare_dram_parameter("bo2", [128, H], f32, isOutput=False)
    out_d = nc.declare_dram_parameter("out", [NB, S, H], f32, isOutput=True)

    with tile.TileContext(nc) as tc:
        from contextlib import ExitStack

        with ExitStack() as ctx:
            ep = ctx.enter_context
            wqk_p = ep(tc.tile_pool(name="wqk", bufs=1))
            wv_p = ep(tc.tile_pool(name="wv", bufs=1))
            wo_p = ep(tc.tile_pool(name="wo", bufs=1))
            x_p = ep(tc.tile_pool(name="xp", bufs=2))
            qk_p = ep(tc.tile_pool(name="qk", bufs=2))
            v_p = ep(tc.tile_pool(name="vp", bufs=2))
            pt_p = ep(tc.tile_pool(name="pt", bufs=8))
            yb_p = ep(tc.tile_pool(name="yb", bufs=2))
            rc_p = ep(tc.tile_pool(name="rc", bufs=4))
            tm_p = ep(tc.tile_pool(name="tm", bufs=3))
            cb_p = ep(tc.tile_pool(name="cb", bufs=1))
            pj_ps = ep(tc.tile_pool(name="pj", bufs=2, space="PSUM"))
            sc_ps = ep(tc.tile_pool(name="sc", bufs=2, space="PSUM"))
            ya_ps = ep(tc.tile_pool(name="ya", bufs=2, space="PSUM"))

            # ---- constants: no DMA needed for the ones row ----
            on_t = cb_p.tile([1, 128], cdt, tag="ones", name="on_t")
            nc.gpsimd.memset(on_t[:], 1.0)
            bqk_t = cb_p.tile([128, 2 * KC], f32, tag="bqk", name="bqk_t")
            bo2_t = cb_p.tile([128, H], f32, tag="bo2", name="bo2_t")
            bo2b_t = cb_p.tile([1, H], cdt, tag="bo2b", name="bo2b_t")

            # ---- head DMA plan: explicit per-queue FIFOs; arrival order is
            # (x(b0) + q,k weight blocks) -> v weights -> x(b1) -> wo ----
            x_t = {0: [None] * KC, 1: [None] * KC}

            def x_dma(b, k, q):
                t = x_p.tile([128, S], cdt, tag=f"x{k}", name=f"x{b}_{k}")
                q.dma_start(out=t[:], in_=xt_d.ap()[b, 128 * k:128 * (k + 1), :])
                x_t[b][k] = t

            wqk_t = [None] * (2 * KC)

            def wqk_dma(ob, q):
                t = wqk_p.tile([128, KC * 128], cdt, tag=f"wqk{ob}", name=f"wqk{ob}")
                q.dma_start(out=t[:], in_=wqk_d.ap()[ob])
                wqk_t[ob] = t

            wv_t = wv_p.tile([128, KC * H], cdt, tag="wv", name="wv_t")

            def wv_dma(j, q):
                w3 = KC * H // 3
                q.dma_start(out=wv_t[:, j * w3:(j + 1) * w3],
                            in_=wv_d.ap()[:, j * w3:(j + 1) * w3])

            wo_t = [None] * KC

            def wo_dma(hb, q):
                t = wo_p.tile([128, H], cdt, tag=f"wo{hb}", name=f"wo{hb}")
                q.dma_start(out=t[:], in_=wo_d.ap()[128 * hb:128 * (hb + 1), :])
                wo_t[hb] = t

            # per-queue FIFOs sequenced against the consumption deadlines of
            # the ob-loop (first MM needs x(b0)+wqk0; wqk[ob] by ~1.3us*ob)
            sy, sc, gp = nc.sync, nc.scalar, nc.gpsimd
            plan = [
                lambda: x_dma(0, 0, sy),
                lambda: wqk_dma(0, sc),
                lambda: x_dma(0, 2, gp),
                lambda: x_dma(0, 1, sy),
                lambda: x_dma(0, 3, sc),
                lambda: x_dma(0, 5, gp),
                lambda: x_dma(0, 4, gp),
                lambda: gp.dma_start(out=bqk_t[:], in_=bqk_d.ap()),
                lambda: wqk_dma(1, sy),
                lambda: wqk_dma(2, sc),
                lambda: wqk_dma(3, gp),
                lambda: wqk_dma(4, sy),
                lambda: wqk_dma(5, sc),
                lambda: wqk_dma(6, gp),
                lambda: wqk_dma(7, sy),
                lambda: wqk_dma(8, sc),
                lambda: wqk_dma(9, gp),
                lambda: wqk_dma(10, sy),
                lambda: wqk_dma(11, sc),
                lambda: wv_dma(0, sy),
                lambda: wv_dma(1, sc),
                lambda: wv_dma(2, gp),
                lambda: x_dma(1, 0, sy),
                lambda: x_dma(1, 1, sc),
                lambda: x_dma(1, 2, gp),
                lambda: x_dma(1, 3, sy),
                lambda: x_dma(1, 4, sc),
                lambda: x_dma(1, 5, gp),
                lambda: wo_dma(0, sy),
                lambda: wo_dma(1, sc),
                lambda: wo_dma(2, gp),
                lambda: wo_dma(3, sy),
                lambda: wo_dma(4, sc),
                lambda: wo_dma(5, gp),
                lambda: gp.dma_start(out=bo2_t[:], in_=bo2_d.ap()),
            ]
            for emit in plan:
                emit()
            nc.vector.tensor_copy(out=bo2b_t[:], in_=bo2_t[0:1, :])

            # ---- building blocks ----
            qk_store = {}

            def qk_chunk(b, ob):
                def emit():
                    ps = pj_ps.tile([128, S], f32, tag="pj", name="pj_ps_t")
                    xt = x_t[b]
                    for j, k in enumerate(KORD):
                        nc.tensor.matmul(
                            ps[:],
                            lhsT=wqk_t[ob][:, 128 * k:128 * (k + 1)],
                            rhs=xt[k][:],
                            start=(j == 0), stop=(j == KC - 1),
                        )
                    t = qk_p.tile([128, S], cdt, tag=f"qk{ob}", name=f"qk{b}_{ob}")
                    nc.vector.tensor_scalar_add(out=t[:], in0=ps[:],
                                                scalar1=bqk_t[:, ob:ob + 1])
                    qk_store[(b, ob)] = t
                return emit

            v_store = {}

            def v_chunk(b, sb, og):
                def emit():
                    if og == 0:
                        vt = v_p.tile([128, NH * VW], cdt, tag=f"v{sb}", name=f"v{b}_{sb}")
                        # upper 64 cols of each head slot must be 1.0 (the
                        # softmax-denominator columns); set the whole tile and
                        # let the copies below overwrite the v columns
                        nc.gpsimd.memset(vt[:], 1.0)
                        v_store[(b, sb)] = vt
                    vt = v_store[(b, sb)]
                    o0, w = (0, 512) if og == 0 else (512, 256)
                    xt = x_t[b]
                    ps = pj_ps.tile([128, S], f32, tag="pj", name="pj_ps_t")
                    for k in range(KC):
                        nc.tensor.matmul(
                            ps[:, :w],
                            lhsT=xt[k][:, 128 * sb:128 * (sb + 1)],
                            rhs=wv_t[:, H * k + o0:H * k + o0 + w],
                            start=(k == 0), stop=(k == KC - 1),
                        )
                    nh = w // DK
                    h0 = o0 // DK
                    src = ps[:, :w].rearrange("p (h c) -> p h c", h=nh)
                    dst = vt[:].rearrange("p (h c) -> p h c", h=NH)[:, h0:h0 + nh, 0:DK]
                    nc.vector.tensor_copy(out=dst, in_=src)
                return emit

            out_q = [0]

            def fproj(b, sb, o0, w, yb_list, engs, act_copy=False):
                st = {}

                def emit_a():
                    ps = pj_ps.tile([128, 512], f32, tag="pj", name="pj_ps_t")
                    st["ps"] = ps
                    for hb in range(KC - 1):
                        nc.tensor.matmul(
                            ps[:, :w],
                            lhsT=yb_list[hb][:, 128 * sb:128 * (sb + 1)],
                            rhs=wo_t[hb][:, o0:o0 + w],
                            start=(hb == 0), stop=False,
                        )

                def emit_b():
                    ps = st["ps"]
                    nc.tensor.matmul(
                        ps[:, :w],
                        lhsT=yb_list[KC - 1][:, 128 * sb:128 * (sb + 1)],
                        rhs=wo_t[KC - 1][:, o0:o0 + w],
                        start=False, stop=True,
                    )
                    ot = tm_p.tile([128, 512], f32, tag="ot", name="ot")
                    nc.vector.tensor_add(out=ot[:, :w], in0=ps[:, :w],
                                         in1=bo2_t[:, o0:o0 + w])
                    eng = engs[out_q[0] % len(engs)]
                    out_q[0] += 1
                    eng.dma_start(
                        out=out_d.ap()[b, 128 * sb:128 * (sb + 1), o0:o0 + w],
                        in_=ot[:, :w],
                    )
                return emit_a, emit_b

            def attention(b, pending, late_pending=None, yb_out=None,
                          kb_pops=(1, 3)):
                """Head-pair attention for batch b; pops `pending` PE-filler
                closures into the ACT-bound gaps. `late_pending` items may
                depend on every pair of this batch but the last, so they only
                pop after the last pair's AV (where they bridge the final
                normalize chain)."""
                late_pending = late_pending or []
                yb_t = [yb_p.tile([128, S], cdt, tag=f"yb{hb}", name=f"yb{b}_{hb}")
                        for hb in range(KC)]
                if yb_out is not None:
                    yb_out.extend(yb_t)
                for hp in range(NH // 2):
                    last = hp == NH // 2 - 1
                    pair = (2 * hp, 2 * hp + 1)
                    q_tile = qk_store[(b, hp)]
                    k_tile = qk_store[(b, KC + hp)]
                    pts = {h: [] for h in pair}
                    for kb in range(SBLK):
                        scp = sc_ps.tile([128, 2 * S], f32, tag="sc", name="sc_ps_t")
                        for hi, h in enumerate(pair):
                            krow = (h % 2) * DK
                            nc.tensor.matmul(
                                scp[:, hi * S:(hi + 1) * S],
                                lhsT=k_tile[krow:krow + DK, 128 * kb:128 * (kb + 1)],
                                rhs=q_tile[krow:krow + DK, :],
                                start=True, stop=True,
                            )
                        ptt = pt_p.tile([128, 2 * S], cdt, tag="ptt", name="ptt")
                        nc.scalar.activation(out=ptt[:], in_=scp[:], func=EXP,
                                             scale=float(1.0 / np.sqrt(DK)))
                        for hi, h in enumerate(pair):
                            pts[h].append(ptt[:, hi * S:(hi + 1) * S])
                        if kb in kb_pops and pending:
                            pending.pop(0)()
                    yps = {h: ya_ps.tile([128, S], f32, tag="ya", name="ya_ps_t")
                           for h in pair}
                    for kb in range(SBLK):
                        for h in pair:
                            nc.tensor.matmul(
                                yps[h][:],
                                lhsT=v_store[(b, kb)][:, VW * h:VW * (h + 1)],
                                rhs=pts[h][kb][:],
                                start=(kb == 0), stop=(kb == SBLK - 1),
                            )
                    if last:
                        while late_pending:
                            late_pending.pop(0)()
                    elif pending:
                        pending.pop(0)()
                    # PSUM rows 64..127 of each head's AV tile hold the
                    # softmax denominator broadcast across 64 partitions
                    den = rc_p.tile([128, S], f32, tag="rec0", name="den")
                    for hi, h in enumerate(pair):
                        nc.vector.tensor_copy(out=den[hi * DK:(hi + 1) * DK, :],
                                              in_=yps[h][DK:2 * DK, :])
                    rec = rc_p.tile([128, S], f32, tag="rec1", name="rec")
                    nc.vector.reciprocal_approx_fast(out=rec[:], in_=den[:])
                    for hi, h in enumerate(pair):
                        krow = hi * DK
                        nc.vector.tensor_mul(out=yb_t[hp][krow:krow + DK, :],
                                             in0=yps[h][0:DK, :],
                                             in1=rec[krow:krow + DK, :])
                while pending:
                    pending.pop(0)()
                while late_pending:
                    late_pending.pop(0)()
                return yb_t

            # ---- batch 0: QKV projection (DMA-paced head phase) ----
            for ob in range(2 * KC):
                qk_chunk(0, ob)()
            for sb in range(SBLK):
                for og in range(2):
                    v_chunk(0, sb, og)()

            # ---- attention(0), filled with QKV(1); defer batch 1's last
            # head-pair q/k blocks into attention(1) for ACT/PE balance ----
            pend0 = []
            for ob in range(2 * KC):
                if ob in (KC - 1, 2 * KC - 1):
                    continue
                pend0.append(qk_chunk(1, ob))
            for sb in range(SBLK):
                for og in range(2):
                    pend0.append(v_chunk(1, sb, og))
            yb0 = attention(0, pend0)

            # ---- attention(1), filled with deferred q/k blocks + fproj(0);
            # the first two drain chunks' A passes ride along at the end so
            # the PE stays busy (and HAM warm) through the last pair's
            # normalize chain ----
            # drain chunks are one whole s-block each: both output column
            # groups accumulate in a single (dead-by-now) 2-bank score-pool
            # PSUM tile, so the A/B pipeline runs at depth 2 s-blocks =
            # 4.8us of pair-5-independent matmul ahead of the first B pass
            yb1_holder = []
            drain = []

            def drain_fproj(sb, act_copy, split_dma, pool):
                st = {}

                def group(ps, o0, w, phase):
                    if phase == "a":
                        if split_dma:
                            # the tail unit keeps the bias matmul so its
                            # PSUM->SBUF step stays a copy (split across
                            # ACT+DVE in parallel)
                            nc.tensor.matmul(ps[:, :w], lhsT=on_t[:],
                                             rhs=bo2b_t[:, o0:o0 + w],
                                             start=True, stop=False)
                        for hb in range(KC - 1):
                            nc.tensor.matmul(
                                ps[:, :w],
                                lhsT=yb1_holder[hb][:, 128 * sb:128 * (sb + 1)],
                                rhs=wo_t[hb][:, o0:o0 + w],
                                start=(hb == 0 and not split_dma), stop=False)
                    else:
                        nc.tensor.matmul(
                            ps[:, :w],
                            lhsT=yb1_holder[KC - 1][:, 128 * sb:128 * (sb + 1)],
                            rhs=wo_t[KC - 1][:, o0:o0 + w],
                            start=False, stop=True)

                def emit_a():
                    if pool == "sc":
                        ps = sc_ps.tile([128, 2 * S], f32, tag="sc", name="sc_ps_t")
                        st["ps"] = [ps[:, 0:512], ps[:, 512:768]]
                    else:
                        p1 = pj_ps.tile([128, 512], f32, tag="pj", name="pj_ps_t")
                        p2 = pj_ps.tile([128, 512], f32, tag="pj", name="pj_ps_t")
                        st["ps"] = [p1[:, :], p2[:, 0:256]]
                    for (o0, w), ps in zip(((0, 512), (512, 256)), st["ps"]):
                        group(ps, o0, w, "a")

                def emit_b():
                    for (o0, w), ps in zip(((0, 512), (512, 256)), st["ps"]):
                        group(ps, o0, w, "b")
                    ot = tm_p.tile([128, H], f32, tag="od", name="od")
                    row = out_d.ap()[1, 128 * sb:128 * (sb + 1), :]
                    if split_dma:
                        # parallel ACT/DVE copies + 3-way DMA so the final
                        # store chain is as short as possible
                        nc.scalar.copy(out=ot[:, 0:256], in_=st["ps"][0][:, 0:256])
                        nc.vector.tensor_copy(out=ot[:, 256:512],
                                              in_=st["ps"][0][:, 256:512])
                        nc.vector.tensor_copy(out=ot[:, 512:768],
                                              in_=st["ps"][1][:, :256])
                        nc.gpsimd.dma_start(out=row[:, 0:256], in_=ot[:, 0:256])
                        nc.sync.dma_start(out=row[:, 256:512], in_=ot[:, 256:512])
                        nc.scalar.dma_start(out=row[:, 512:768], in_=ot[:, 512:768])
                        return
                    nc.vector.tensor_add(out=ot[:, :512], in0=st["ps"][0],
                                         in1=bo2_t[:, 0:512])
                    nc.vector.tensor_add(out=ot[:, 512:768], in0=st["ps"][1],
                                         in1=bo2_t[:, 512:768])
                    eng = [nc.gpsimd, nc.sync, nc.scalar][sb % 3]
                    eng.dma_start(out=row, in_=ot[:])
                return emit_a, emit_b

            for sb in range(SBLK):
                drain.append(drain_fproj(sb, act_copy=(sb % 2 == 0),
                                         split_dma=(sb == SBLK - 1),
                                         pool=("pj" if sb == 2 else "sc")))

            # big/small fproj(0) chunks alternate so every pair gets ~equal
            # filler; the s-block-2 drain A pass (pj pool, independent of the
            # score pool) rides the last pair's kb slot
            pend1 = [qk_chunk(1, KC - 1), qk_chunk(1, 2 * KC - 1)]
            for sb in range(SBLK):
                for (o0, w) in ((0, 512), (512, 256)):
                    ea, eb = fproj(0, sb, o0, w, yb0, engs=[nc.sync, nc.gpsimd])
                    pend1.append(lambda ea=ea, eb=eb: (ea(), eb()))
            pend1.append(drain[2][0])
            attention(1, pend1, late_pending=[drain[0][0], drain[1][0]],
                      yb_out=yb1_holder, kb_pops=(3,))

            # ---- fproj(1) drain: pass B (head 5 + store) interleaved with
            # the remaining pass A at PSUM pipeline depth 2 (A passes for
            # s-blocks 0-2 already ran as late fillers inside attention(1)) ----
            drain[0][1]()
            drain[3][0]()
            drain[1][1]()
            drain[2][1]()
            drain[3][1]()

    nc.compile()
    return nc


def get_program():
    if "nc" not in _PROG_CACHE:
        _PROG_CACHE["nc"] = _build_program()
    return _PROG_CACHE["nc"]


def make_in_maps(x, w_qkv_w, w_qkv_b, w_o_w, w_o_b):
    import ml_dtypes
    np_cdt = ml_dtypes.bfloat16
    x = np.asarray(x, np.float32)
    xT = np.ascontiguousarray(np.transpose(x, (0, 2, 1)).astype(np_cdt))  # [B, H, S]
    wqkvT = np.asarray(w_qkv_w, np.float32).T  # [H, 3H]
    # q,k columns blocked per transposed output block:
    # wqkb[ob][p][k*128+c] = wqkvT[k*128+p, ob*128+c]
    t = wqkvT[:, :2 * H].reshape(KC, 128, 2 * KC, 128)
    wqkb = np.ascontiguousarray(t.transpose(2, 1, 0, 3).reshape(2 * KC, 128, KC * 128).astype(np_cdt))
    # v columns in contraction-chunk-major rows: wvb[p][k*H+c] = wqkvT[k*128+p, 2H+c]
    tv = wqkvT[:, 2 * H:].reshape(KC, 128, H)
    wvb = np.ascontiguousarray(tv.transpose(1, 0, 2).reshape(128, KC * H).astype(np_cdt))
    woT = np.ascontiguousarray(np.asarray(w_o_w, np.float32).T.astype(np_cdt))  # [H, H]
    # qk bias as [128, 12] f32: bqk[p, j] = w_qkv_b[j*128+p]
    bqk = np.ascontiguousarray(
        np.asarray(w_qkv_b, np.float32)[:2 * H].reshape(2 * KC, 128).T)
    # v-projection bias folded into the output bias (y = Sum p (v0+bv)/Sum p
    # = y0 + bv, so out = y0 @ w_o^T + (bo + w_o @ bv)), replicated across
    # partitions so the PSUM->SBUF step applies it as a tensor_add
    bv_f = np.asarray(w_qkv_b, np.float32)[2 * H:]
    bo2_row = np.asarray(w_o_b, np.float32) + np.asarray(w_o_w, np.float32) @ bv_f
    bo2 = np.ascontiguousarray(np.broadcast_to(bo2_row, (128, H)).astype(np.float32))
    return [
        {
            "xt": np.ascontiguousarray(xT[NB * c:NB * (c + 1)]),
            "wqkb": wqkb,
            "wvb": wvb,
            "wot": woT,
            "bqk": bqk,
            "bo2": bo2,
        }
        for c in range(N_CORES)
    ]


def _numpy_fallback(x, attn_mask, w_qkv_w, w_qkv_b, w_o_w, w_o_b):
    x = np.asarray(x, np.float64)
    qkv = x @ np.asarray(w_qkv_w, np.float64).T + np.asarray(w_qkv_b, np.float64)
    q, k, v = np.split(qkv, 3, axis=-1)

    def heads(t):
        return t.reshape(B, S, NH, DK).transpose(0, 2, 1, 3)

    q, k, v = heads(q), heads(k), heads(v)
    s = np.einsum("bhqd,bhkd->bhqk", q, k) / np.sqrt(DK)
    mask = np.asarray(attn_mask, bool)[:, None, None, :]
    s = np.where(mask, s, -np.inf)
    s = s - s.max(axis=-1, keepdims=True)
    p = np.exp(s)
    p = p / p.sum(axis=-1, keepdims=True)
    y = np.einsum("bhqk,bhkd->bhqd", p, v)
    y = y.transpose(0, 2, 1, 3).reshape(B, S, H)
    out = y @ np.asarray(w_o_w, np.float64).T + np.asarray(w_o_b, np.float64)
    return out.astype(np.float32)


def kernel(x, attn_mask, w_qkv_w, w_qkv_b, w_o_w, w_o_b):
    if not bool(np.all(np.asarray(attn_mask))):
        return _numpy_fallback(x, attn_mask, w_qkv_w, w_qkv_b, w_o_w, w_o_b)

    from concourse.bass_utils import run_bass_kernel_spmd

    nc = get_program()
    in_maps = make_in_maps(x, w_qkv_w, w_qkv_b, w_o_w, w_o_b)
    res = run_bass_kernel_spmd(nc, in_maps, list(range(N_CORES)))
    out = np.concatenate([res.results[c]["out"] for c in range(N_CORES)], axis=0)
    return out.astype(np.float32)


# revision 44
# speedup vs baseline: 1.0469x; 1.0469x over previous
"""Multi-head attention (B=16, S=512, H=768, NH=12) on 8 Trainium2 NeuronCores.

Strategy: data-parallel over batch - 2 batches per core, no collectives.

v2 dataflow (all matmul inputs bf16, fp32 PSUM accumulation). The kernel is
PE-bound (~95us of matmul at 2.4GHz per core), so the structure keeps the PE
issue queue dense from the first microsecond after the runtime preamble:

  - wqkv^T for q,k is pre-blocked host-side into 12 column blocks
    [128, 6*128] (one per transposed output block) so each block is a single
    contiguous 0.2MB DMA; blocks stream round-robin across the 3 DGE queues
    (sync/scalar/gpsimd) interleaved with the x chunks, and the first
    projection matmul issues ~1us after the DMA engines come up.
  - QKV projection for q,k computed transposed (qkv^T[o, s]) so per-head
    q^T/k^T land with the head dim on partitions; v in natural [s, o]
    orientation into per-head slots of width 128 whose upper 64 columns are
    ones (tile is memset to 1.0, then the v columns are overwritten) so the
    attention-value matmul also emits the softmax denominator.
  - scores^T = k^T.T @ q^T per head pair into one 2-bank PSUM tile; heads of
    a pair occupy PE row-groups 0-63/64-127 so their matmuls run
    concurrently; one wide exp per (pair, kb) on ScalarE with the 1/sqrt(dk)
    scale fused (no max-subtraction: |scores| < ~10 for these activations).
  - AV accumulates over the 4 sk blocks; PSUM rows 64..127 hold the
    denominator broadcast over 64 partitions; per-head reciprocal reads the
    denominator rows straight out of PSUM, then one multiply per head writes
    the normalized y^T block.
  - attention is ACT(exp)-bound, so the PE idle inside attention(b0) is
    filled with the whole QKV projection of batch 1, and attention(b1) is
    filled with batch 0's output projection; the q/k blocks of batch 1's
    last head pair are deferred into attention(b1) to balance it. The final
    output projection drains in two passes (heads 0-4 first, then head 5 +
    bias + store) so it overlaps the last pair's normalize chain.

attn_mask from the reference setup is all-ones; a non-trivial mask falls
back to a numpy implementation.
"""

import sys

sys.path.insert(0, "/opt/trn_rl_repo")

import numpy as np

B, S, H, NH = 16, 512, 768, 12
DK = H // NH  # 64
N_CORES = 8
NB = B // N_CORES  # batches per core = 2
KC = H // 128  # 6 contraction chunks
SBLK = S // 128  # 4 s-blocks of 128
VW = 2 * DK  # 128: per-head v slot width (64 v cols + 64 ones cols)
NORM_SAFE = True  # True: baseline den-gather normalize (more DVE time)
KORD = [0, 2, 5, 1, 3, 4]  # contraction order matched to x-chunk DMA arrival

_PROG_CACHE = {}


def _build_program():
    import concourse.tile as tile
    from concourse import bacc, mybir

    f32 = mybir.dt.float32
    cdt = mybir.dt.bfloat16
    EXP = mybir.ActivationFunctionType.Exp

    nc = bacc.Bacc("TRN2", target_bir_lowering=False, debug=False,
                   num_devices=N_CORES)

    xt_d = nc.declare_dram_parameter("xt", [NB, H, S], cdt, isOutput=False)
    wqk_d = nc.declare_dram_parameter("wqkb", [2 * KC, 128, KC * 128], cdt, isOutput=False)
    wv_d = nc.declare_dram_parameter("wvb", [128, KC * H], cdt, isOutput=False)
    wo_d = nc.declare_dram_parameter("wot", [H, H], cdt, isOutput=False)
    bqk_d = nc.declare_dram_parameter("bqk", [128, 2 * KC], f32, isOutput=False)
    bo2_d = nc.declare_dram_parameter("bo2", [1, H], cdt, isOutput=False)
    out_d = nc.declare_dram_parameter("out", [NB, S, H], f32, isOutput=True)

    with tile.TileContext(nc) as tc:
        from contextlib import ExitStack

        with ExitStack() as ctx:
            ep = ctx.enter_context
            wqk_p = ep(tc.tile_pool(name="wqk", bufs=1))
            wv_p = ep(tc.tile_pool(name="wv", bufs=1))
            wo_p = ep(tc.tile_pool(name="wo", bufs=1))
            x_p = ep(tc.tile_pool(name="xp", bufs=2))
            qk_p = ep(tc.tile_pool(name="qk", bufs=2))
            v_p = ep(tc.tile_pool(name="vp", bufs=2))
            pt_p = ep(tc.tile_pool(name="pt", bufs=8))
            yb_p = ep(tc.tile_pool(name="yb", bufs=2))
            rc_p = ep(tc.tile_pool(name="rc", bufs=4))
            tm_p = ep(tc.tile_pool(name="tm", bufs=3))
            cb_p = ep(tc.tile_pool(name="cb", bufs=1))
            pj_ps = ep(tc.tile_pool(name="pj", bufs=2, space="PSUM"))
            sc_ps = ep(tc.tile_pool(name="sc", bufs=2, space="PSUM"))
            ya_ps = ep(tc.tile_pool(name="ya", bufs=2, space="PSUM"))

            # ---- constants: no DMA needed for the ones row ----
            on_t = cb_p.tile([1, 128], cdt, tag="ones", name="on_t")
            nc.gpsimd.memset(on_t[:], 1.0)
            bqk_t = cb_p.tile([128, 2 * KC], f32, tag="bqk", name="bqk_t")
            bo2b_t = cb_p.tile([1, H], cdt, tag="bo2b", name="bo2b_t")

            # ---- head DMA plan: explicit per-queue FIFOs; arrival order is
            # (x(b0) + q,k weight blocks) -> v weights -> x(b1) -> wo ----
            x_t = {0: [None] * KC, 1: [None] * KC}

            def x_dma(b, k, q):
                t = x_p.tile([128, S], cdt, tag=f"x{k}", name=f"x{b}_{k}")
                q.dma_start(out=t[:], in_=xt_d.ap()[b, 128 * k:128 * (k + 1), :])
                x_t[b][k] = t

            wqk_t = [None] * (2 * KC)

            def wqk_dma(ob, q):
                t = wqk_p.tile([128, KC * 128], cdt, tag=f"wqk{ob}", name=f"wqk{ob}")
                q.dma_start(out=t[:], in_=wqk_d.ap()[ob])
                wqk_t[ob] = t

            wv_t = wv_p.tile([128, KC * H], cdt, tag="wv", name="wv_t")

            def wv_dma(j, q):
                w3 = KC * H // 3
                q.dma_start(out=wv_t[:, j * w3:(j + 1) * w3],
                            in_=wv_d.ap()[:, j * w3:(j + 1) * w3])

            wo_t = [None] * KC

            def wo_dma(hb, q):
                t = wo_p.tile([128, H], cdt, tag=f"wo{hb}", name=f"wo{hb}")
                q.dma_start(out=t[:], in_=wo_d.ap()[128 * hb:128 * (hb + 1), :])
                wo_t[hb] = t

            # per-queue FIFOs sequenced against the consumption deadlines of
            # the ob-loop (first MM needs x(b0)+wqk0; wqk[ob] by ~1.3us*ob)
            sy, sc, gp = nc.sync, nc.scalar, nc.gpsimd
            plan = [
                lambda: x_dma(0, 0, sy),
                lambda: wqk_dma(0, sc),
                lambda: x_dma(0, 2, gp),
                lambda: x_dma(0, 1, sy),
                lambda: x_dma(0, 3, sc),
                lambda: x_dma(0, 5, gp),
                lambda: x_dma(0, 4, gp),
                lambda: gp.dma_start(out=bqk_t[:], in_=bqk_d.ap()),
                lambda: wqk_dma(1, sy),
                lambda: wqk_dma(2, sc),
                lambda: wqk_dma(3, gp),
                lambda: wqk_dma(4, sy),
                lambda: wqk_dma(5, sc),
                lambda: wqk_dma(6, gp),
                lambda: wqk_dma(7, sy),
                lambda: wqk_dma(8, sc),
                lambda: wqk_dma(9, gp),
                lambda: wqk_dma(10, sy),
                lambda: wqk_dma(11, sc),
                lambda: wv_dma(0, sy),
                lambda: wv_dma(1, sc),
                lambda: wv_dma(2, gp),
                lambda: x_dma(1, 0, sy),
                lambda: x_dma(1, 1, sc),
                lambda: x_dma(1, 2, gp),
                lambda: x_dma(1, 3, sy),
                lambda: x_dma(1, 4, sc),
                lambda: x_dma(1, 5, gp),
                lambda: wo_dma(0, sy),
                lambda: wo_dma(1, sc),
                lambda: wo_dma(2, gp),
                lambda: wo_dma(3, sy),
                lambda: wo_dma(4, sc),
                lambda: wo_dma(5, gp),
                lambda: gp.dma_start(out=bo2b_t[:], in_=bo2_d.ap()),
            ]
            for emit in plan:
                emit()

            # ---- building blocks ----
            qk_store = {}

            def qk_chunk(b, ob):
                def emit():
                    ps = pj_ps.tile([128, S], f32, tag="pj", name="pj_ps_t")
                    xt = x_t[b]
                    for j, k in enumerate(KORD):
                        nc.tensor.matmul(
                            ps[:],
                            lhsT=wqk_t[ob][:, 128 * k:128 * (k + 1)],
                            rhs=xt[k][:],
                            start=(j == 0), stop=(j == KC - 1),
                        )
                    t = qk_p.tile([128, S], cdt, tag=f"qk{ob}", name=f"qk{b}_{ob}")
                    nc.vector.tensor_scalar_add(out=t[:], in0=ps[:],
                                                scalar1=bqk_t[:, ob:ob + 1])
                    qk_store[(b, ob)] = t
                return emit

            v_store = {}

            def v_chunk(b, sb, og):
                def emit():
                    if og == 0:
                        vt = v_p.tile([128, NH * VW], cdt, tag=f"v{sb}", name=f"v{b}_{sb}")
                        # upper 64 cols of each head slot must be 1.0 (the
                        # softmax-denominator columns); set the whole tile and
                        # let the copies below overwrite the v columns
                        nc.gpsimd.memset(vt[:], 1.0)
                        v_store[(b, sb)] = vt
                    vt = v_store[(b, sb)]
                    o0, w = (0, 512) if og == 0 else (512, 256)
                    xt = x_t[b]
                    ps = pj_ps.tile([128, S], f32, tag="pj", name="pj_ps_t")
                    for k in range(KC):
                        nc.tensor.matmul(
                            ps[:, :w],
                            lhsT=xt[k][:, 128 * sb:128 * (sb + 1)],
                            rhs=wv_t[:, H * k + o0:H * k + o0 + w],
                            start=(k == 0), stop=(k == KC - 1),
                        )
                    nh = w // DK
                    h0 = o0 // DK
                    src = ps[:, :w].rearrange("p (h c) -> p h c", h=nh)
                    dst = vt[:].rearrange("p (h c) -> p h c", h=NH)[:, h0:h0 + nh, 0:DK]
                    nc.vector.tensor_copy(out=dst, in_=src)
                return emit

            out_q = [0]

            def fproj(b, sb, o0, w, yb_list, engs, act_copy=False):
                st = {}

                def emit_a():
                    ps = pj_ps.tile([128, 512], f32, tag="pj", name="pj_ps_t")
                    st["ps"] = ps
                    nc.tensor.matmul(
                        ps[:, :w],
                        lhsT=on_t[:],
                        rhs=bo2b_t[:, o0:o0 + w],
                        start=True, stop=False,
                    )
                    for hb in range(KC - 1):
                        nc.tensor.matmul(
                            ps[:, :w],
                            lhsT=yb_list[hb][:, 128 * sb:128 * (sb + 1)],
                            rhs=wo_t[hb][:, o0:o0 + w],
                            start=False, stop=False,
                        )

                def emit_b():
                    ps = st["ps"]
                    nc.tensor.matmul(
                        ps[:, :w],
                        lhsT=yb_list[KC - 1][:, 128 * sb:128 * (sb + 1)],
                        rhs=wo_t[KC - 1][:, o0:o0 + w],
                        start=False, stop=True,
                    )
                    ot = tm_p.tile([128, 512], f32, tag="ot", name="ot")
                    if act_copy:
                        nc.scalar.copy(out=ot[:, :w], in_=ps[:, :w])
                    else:
                        nc.vector.tensor_copy(out=ot[:, :w], in_=ps[:, :w])
                    eng = engs[out_q[0] % len(engs)]
                    out_q[0] += 1
                    eng.dma_start(
                        out=out_d.ap()[b, 128 * sb:128 * (sb + 1), o0:o0 + w],
                        in_=ot[:, :w],
                    )
                return emit_a, emit_b

            def attention(b, pending, late_pending=None, yb_out=None,
                          kb_pops=(1, 3)):
                """Head-pair attention for batch b; pops `pending` PE-filler
                closures into the ACT-bound gaps. `late_pending` items may
                depend on every pair of this batch but the last, so they only
                pop after the last pair's AV (where they bridge the final
                normalize chain)."""
                late_pending = late_pending or []
                yb_t = [yb_p.tile([128, S], cdt, tag=f"yb{hb}", name=f"yb{b}_{hb}")
                        for hb in range(KC)]
                if yb_out is not None:
                    yb_out.extend(yb_t)
                for hp in range(NH // 2):
                    last = hp == NH // 2 - 1
                    pair = (2 * hp, 2 * hp + 1)
                    q_tile = qk_store[(b, hp)]
                    k_tile = qk_store[(b, KC + hp)]
                    pts = {h: [] for h in pair}
                    for kb in range(SBLK):
                        scp = sc_ps.tile([128, 2 * S], f32, tag="sc", name="sc_ps_t")
                        for hi, h in enumerate(pair):
                            krow = (h % 2) * DK
                            nc.tensor.matmul(
                                scp[:, hi * S:(hi + 1) * S],
                                lhsT=k_tile[krow:krow + DK, 128 * kb:128 * (kb + 1)],
                                rhs=q_tile[krow:krow + DK, :],
                                start=True, stop=True,
                            )
                        ptt = pt_p.tile([128, 2 * S], cdt, tag="ptt", name="ptt")
                        nc.scalar.activation(out=ptt[:], in_=scp[:], func=EXP,
                                             scale=float(1.0 / np.sqrt(DK)))
                        for hi, h in enumerate(pair):
                            pts[h].append(ptt[:, hi * S:(hi + 1) * S])
                        if kb in kb_pops and pending:
                            pending.pop(0)()
                    yps = {h: ya_ps.tile([128, S], f32, tag="ya", name="ya_ps_t")
                           for h in pair}
                    for kb in range(SBLK):
                        for h in pair:
                            nc.tensor.matmul(
                                yps[h][:],
                                lhsT=v_store[(b, kb)][:, VW * h:VW * (h + 1)],
                                rhs=pts[h][kb][:],
                                start=(kb == 0), stop=(kb == SBLK - 1),
                            )
                    if last:
                        while late_pending:
                            late_pending.pop(0)()
                    elif pending:
                        pending.pop(0)()
                    # PSUM rows 64..127 of each head's AV tile hold the
                    # softmax denominator broadcast across 64 partitions
                    den = rc_p.tile([128, S], f32, tag="rec0", name="den")
                    for hi, h in enumerate(pair):
                        nc.vector.tensor_copy(out=den[hi * DK:(hi + 1) * DK, :],
                                              in_=yps[h][DK:2 * DK, :])
                    rec = rc_p.tile([128, S], f32, tag="rec1", name="rec")
                    nc.vector.reciprocal_approx_fast(out=rec[:], in_=den[:])
                    for hi, h in enumerate(pair):
                        krow = hi * DK
                        nc.vector.tensor_mul(out=yb_t[hp][krow:krow + DK, :],
                                             in0=yps[h][0:DK, :],
                                             in1=rec[krow:krow + DK, :])
                while pending:
                    pending.pop(0)()
                while late_pending:
                    late_pending.pop(0)()
                return yb_t

            # ---- batch 0: QKV projection (DMA-paced head phase) ----
            for ob in range(2 * KC):
                qk_chunk(0, ob)()
            for sb in range(SBLK):
                for og in range(2):
                    v_chunk(0, sb, og)()

            # ---- attention(0), filled with QKV(1); defer batch 1's last
            # head-pair q/k blocks into attention(1) for ACT/PE balance ----
            pend0 = []
            for ob in range(2 * KC):
                if ob in (KC - 1, 2 * KC - 1):
                    continue
                pend0.append(qk_chunk(1, ob))
            for sb in range(SBLK):
                for og in range(2):
                    pend0.append(v_chunk(1, sb, og))
            yb0 = attention(0, pend0)

            # ---- attention(1), filled with deferred q/k blocks + fproj(0);
            # the first two drain chunks' A passes ride along at the end so
            # the PE stays busy (and HAM warm) through the last pair's
            # normalize chain ----
            # drain chunks are one whole s-block each: both output column
            # groups accumulate in a single (dead-by-now) 2-bank score-pool
            # PSUM tile, so the A/B pipeline runs at depth 2 s-blocks =
            # 4.8us of pair-5-independent matmul ahead of the first B pass
            yb1_holder = []
            drain = []

            def drain_fproj(sb, act_copy, split_dma, pool):
                st = {}

                def group(ps, o0, w, phase):
                    if phase == "a":
                        nc.tensor.matmul(ps[:, :w], lhsT=on_t[:],
                                         rhs=bo2b_t[:, o0:o0 + w],
                                         start=True, stop=False)
                        for hb in range(KC - 1):
                            nc.tensor.matmul(
                                ps[:, :w],
                                lhsT=yb1_holder[hb][:, 128 * sb:128 * (sb + 1)],
                                rhs=wo_t[hb][:, o0:o0 + w],
                                start=False, stop=False)
                    else:
                        nc.tensor.matmul(
                            ps[:, :w],
                            lhsT=yb1_holder[KC - 1][:, 128 * sb:128 * (sb + 1)],
                            rhs=wo_t[KC - 1][:, o0:o0 + w],
                            start=False, stop=True)

                def emit_a():
                    if pool == "sc":
                        ps = sc_ps.tile([128, 2 * S], f32, tag="sc", name="sc_ps_t")
                        st["ps"] = [ps[:, 0:512], ps[:, 512:768]]
                    else:
                        p1 = pj_ps.tile([128, 512], f32, tag="pj", name="pj_ps_t")
                        p2 = pj_ps.tile([128, 512], f32, tag="pj", name="pj_ps_t")
                        st["ps"] = [p1[:, :], p2[:, 0:256]]
                    for (o0, w), ps in zip(((0, 512), (512, 256)), st["ps"]):
                        group(ps, o0, w, "a")

                def emit_b():
                    for (o0, w), ps in zip(((0, 512), (512, 256)), st["ps"]):
                        group(ps, o0, w, "b")
                    ot = tm_p.tile([128, H], f32, tag="od", name="od")
                    row = out_d.ap()[1, 128 * sb:128 * (sb + 1), :]
                    if split_dma:
                        # parallel ACT/DVE copies + 3-way DMA so the final
                        # store chain is as short as possible
                        nc.scalar.copy(out=ot[:, 0:256], in_=st["ps"][0][:, 0:256])
                        nc.vector.tensor_copy(out=ot[:, 256:512],
                                              in_=st["ps"][0][:, 256:512])
                        nc.vector.tensor_copy(out=ot[:, 512:768],
                                              in_=st["ps"][1][:, :256])
                        nc.gpsimd.dma_start(out=row[:, 0:256], in_=ot[:, 0:256])
                        nc.sync.dma_start(out=row[:, 256:512], in_=ot[:, 256:512])
                        nc.scalar.dma_start(out=row[:, 512:768], in_=ot[:, 512:768])
                        return
                    if act_copy:
                        nc.scalar.copy(out=ot[:, :512], in_=st["ps"][0])
                        nc.scalar.copy(out=ot[:, 512:768], in_=st["ps"][1])
                    else:
                        nc.vector.tensor_copy(out=ot[:, :512], in_=st["ps"][0])
                        nc.vector.tensor_copy(out=ot[:, 512:768], in_=st["ps"][1])
                    eng = [nc.gpsimd, nc.sync, nc.scalar][sb % 3]
                    eng.dma_start(out=row, in_=ot[:])
                return emit_a, emit_b

            for sb in range(SBLK):
                drain.append(drain_fproj(sb, act_copy=(sb % 2 == 0),
                                         split_dma=(sb == SBLK - 1),
                                         pool=("pj" if sb == 2 else "sc")))

            # big/small fproj(0) chunks alternate so every pair gets ~equal
            # filler; the s-block-2 drain A pass (pj pool, independent of the
            # score pool) rides the last pair's kb slot
            pend1 = [qk_chunk(1, KC - 1), qk_chunk(1, 2 * KC - 1)]
            for sb in range(SBLK):
                for (o0, w) in ((0, 512), (512, 256)):
                    ea, eb = fproj(0, sb, o0, w, yb0, engs=[nc.sync, nc.gpsimd])
                    pend1.append(lambda ea=ea, eb=eb: (ea(), eb()))
            pend1.append(drain[2][0])
            attention(1, pend1, late_pending=[drain[0][0], drain[1][0]],
                      yb_out=yb1_holder, kb_pops=(3,))

            # ---- fproj(1) drain: pass B (head 5 + store) interleaved with
            # the remaining pass A at PSUM pipeline depth 2 (A passes for
            # s-blocks 0-2 already ran as late fillers inside attention(1)) ----
            drain[0][1]()
            drain[3][0]()
            drain[1][1]()
            drain[2][1]()
            drain[3][1]()

    nc.compile()
    return nc


def get_program():
    if "nc" not in _PROG_CACHE:
        _PROG_CACHE["nc"] = _build_program()
    return _PROG_CACHE["nc"]


def make_in_maps(x, w_qkv_w, w_qkv_b, w_o_w, w_o_b):
    import ml_dtypes
    np_cdt = ml_dtypes.bfloat16
    x = np.asarray(x, np.float32)
    xT = np.ascontiguousarray(np.transpose(x, (0, 2, 1)).astype(np_cdt))  # [B, H, S]
    wqkvT = np.asarray(w_qkv_w, np.float32).T  # [H, 3H]
    # q,k columns blocked per transposed output block:
    # wqkb[ob][p][k*128+c] = wqkvT[k*128+p, ob*128+c]
    t = wqkvT[:, :2 * H].reshape(KC, 128, 2 * KC, 128)
    wqkb = np.ascontiguousarray(t.transpose(2, 1, 0, 3).reshape(2 * KC, 128, KC * 128).astype(np_cdt))
    # v columns in contraction-chunk-major rows: wvb[p][k*H+c] = wqkvT[k*128+p, 2H+c]
    tv = wqkvT[:, 2 * H:].reshape(KC, 128, H)
    wvb = np.ascontiguousarray(tv.transpose(1, 0, 2).reshape(128, KC * H).astype(np_cdt))
    woT = np.ascontiguousarray(np.asarray(w_o_w, np.float32).T.astype(np_cdt))  # [H, H]
    # qk bias as [128, 12] f32: bqk[p, j] = w_qkv_b[j*128+p]
    bqk = np.ascontiguousarray(
        np.asarray(w_qkv_b, np.float32)[:2 * H].reshape(2 * KC, 128).T)
    # v-projection bias folded into the output bias (y = Sum p (v0+bv)/Sum p
    # = y0 + bv, so out = y0 @ w_o^T + (bo + w_o @ bv)), replicated across
    # partitions so the PSUM->SBUF step applies it as a tensor_add
    bv_f = np.asarray(w_qkv_b, np.float32)[2 * H:]
    bo2_row = np.asarray(w_o_b, np.float32) + np.asarray(w_o_w, np.float32) @ bv_f
    bo2 = np.ascontiguousarray(bo2_row.reshape(1, H).astype(np_cdt))
    return [
        {
            "xt": np.ascontiguousarray(xT[NB * c:NB * (c + 1)]),
            "wqkb": wqkb,
            "wvb": wvb,
            "wot": woT,
            "bqk": bqk,
            "bo2": bo2,
        }
        for c in range(N_CORES)
    ]


def _numpy_fallback(x, attn_mask, w_qkv_w, w_qkv_b, w_o_w, w_o_b):
    x = np.asarray(x, np.float64)
    qkv = x @ np.asarray(w_qkv_w, np.float64).T + np.asarray(w_qkv_b, np.float64)
    q, k, v = np.split(qkv, 3, axis=-1)

    def heads(t):
        return t.reshape(B, S, NH, DK).transpose(0, 2, 1, 3)

    q, k, v = heads(q), heads(k), heads(v)
    s = np.einsum("bhqd,bhkd->bhqk", q, k) / np.sqrt(DK)
    mask = np.asarray(attn_mask, bool)[:, None, None, :]
    s = np.where(mask, s, -np.inf)
    s = s - s.max(axis=-1, keepdims=True)
    p = np.exp(s)
    p = p / p.sum(axis=-1, keepdims=True)
    y = np.einsum("bhqk,bhkd->bhqd", p, v)
    y = y.transpose(0, 2, 1, 3).reshape(B, S, H)
    out = y @ np.asarray(w_o_w, np.float64).T + np.asarray(w_o_b, np.float64)
    return out.astype(np.float32)


def kernel(x, attn_mask, w_qkv_w, w_qkv_b, w_o_w, w_o_b):
    if not bool(np.all(np.asarray(attn_mask))):
        return _numpy_fallback(x, attn_mask, w_qkv_w, w_qkv_b, w_o_w, w_o_b)

    from concourse.bass_utils import run_bass_kernel_spmd

    nc = get_program()
    in_maps = make_in_maps(x, w_qkv_w, w_qkv_b, w_o_w, w_o_b)
    res = run_bass_kernel_spmd(nc, in_maps, list(range(N_CORES)))
    out = np.concatenate([res.results[c]["out"] for c in range(N_CORES)], axis=0)
    return out.astype(np.float32)
